# revision 62
# baseline (speedup 1.0000x reference)
"""Causal self-attention Trainium2 kernel, 8-core SPMD (head-sharded tensor parallel).

Model: B=4, T=2048, D=1024, H=16 heads x 64. out = softmax(mask(QK^T/8)) V W_proj^T.

Sharding: 2 cores per batch, split on HEADS (tensor parallel). Core c handles
batch c//2 and heads [8*(c%2), 8*(c%2)+8) -- 4 head pairs -- over ALL 2048
tokens. Each core computes Q/K/V projections for its 8 heads only (half the
projection FLOPs of a token split), full causal attention for those heads, and
a PARTIAL output projection out_part = attn_local @ W_proj[:, local].T. The
host sums the two partial projections per batch when unsharding (the tensor-
parallel all-reduce, done at gather time), so the device program needs no
collectives and the SPMD program is identical on all cores.

All matmuls run in bf16 (fp32 accumulate). Scores are pre-scaled by 1/8 via
the Q weights and |score| <= ~3.1, so softmax needs no max subtraction. The
causal staircase is applied POST-exp as a multiplicative 0/1 mask on the DVE.

Scheduling: the attention inner loop is ACT(exp)-latency bound (~860ns/unit vs
~640ns of PE work), so all projection work (V, K/Q of later pairs, output proj
chunks) is chopped into single-matmul "filler" generators and 4 filler matmuls
are pulled between each unit's QK (emitted one unit ahead) and the previous
unit's PV. This keeps the PE queue dense: the PE never sits on a PV waiting
for its exp, and the HAM clock gate stays at 8/8 (a warmup burst on the mask
tile trips it to full rate before the first real matmul).

Per-core PE cycle budget (N-streaming cycles @2.4GHz):
  V 65536 + K 65536 + Q 65536 + QK 69632 (2 heads concurrent via PE row
  groups) + PV 139264 + proj 65536 = 471k cycles ~= 196us.
"""

import os
from collections import deque
from contextlib import ExitStack

import numpy as np
import ml_dtypes

import concourse.bass as bass
import concourse.mybir as mybir
import concourse.tile as tile
from concourse import bacc
from concourse.bass_utils import run_bass_kernel_spmd

BF16 = mybir.dt.bfloat16
F32 = mybir.dt.float32
EXP = mybir.ActivationFunctionType.Exp

B, T, D = 4, 2048, 1024
H, DH = 16, 64
NCORES = 8
KT = 16          # k-tiles of 128 over T
NPAIR = 4        # local head pairs (8 heads per core)
NJ = 4           # 512-wide query chunks over T

_cached = {}

if os.environ.get("BASS_LDW_OPT", "") == "1":
    # A/B experiment only -- walrus crashes on this program with
    # --enable-ldw-opt=true, so this must stay opt-in
    # (the default path passes --enable-ldw-opt=false, which serializes a
    # ~60-140ns weight load in front of nearly every matmul)
    from concourse import bass_utils as _bu
    _orig_run_command = _bu.run_command
    def _patched_run_command(argv, **kwargs):
        argv = [a.replace("--enable-ldw-opt=false", "--enable-ldw-opt=true")
                if isinstance(a, str) else a for a in argv]
        return _orig_run_command(argv, **kwargs)
    _bu.run_command = _patched_run_command


class Filler:
    """FIFO of single-matmul emission generators with drain deadlines."""

    def __init__(self):
        self.q = deque()  # (deadline_key, generator)

    def add(self, deadline, gen):
        self.q.append((deadline, gen))

    def pull(self, n, horizon=(99, 99)):
        got = 0
        while n > 0 and self.q and self.q[0][0] <= horizon:
            _, g = self.q[0]
            try:
                next(g)
                n -= 1
                got += 1
            except StopIteration:
                self.q.popleft()
        return got

    def drain_until(self, key):
        while self.q and self.q[0][0] <= key:
            _, g = self.q[0]
            for _ in g:
                pass
            self.q.popleft()

    def drain_all(self):
        self.drain_until((99, 99))


def _build_program():
    nc = bacc.Bacc("TRN2", name="causal_attn_tp")

    x_T = nc.dram_tensor("x_T", [D, T], BF16, kind="ExternalInput")
    w_qT = nc.dram_tensor("w_qT", [D, 512], BF16, kind="ExternalInput")
    w_kT = nc.dram_tensor("w_kT", [D, 512], BF16, kind="ExternalInput")
    w_vT = nc.dram_tensor("w_vT", [D, 512], BF16, kind="ExternalInput")
    w_pT = nc.dram_tensor("w_pT", [512, D], BF16, kind="ExternalInput")
    maskd = nc.dram_tensor("mask", [128, 256], BF16, kind="ExternalInput")
    identd = nc.dram_tensor("ident", [128, 128], BF16, kind="ExternalInput")
    # bf16 partial output: halves the output DMA and the tail exposure; the
    # host sums the two bf16 partials in fp32 (error well inside budget)
    outd = nc.dram_tensor("out_T", [D, T], BF16, kind="ExternalOutput")

    with ExitStack() as ctx:
        tc = ctx.enter_context(tile.TileContext(nc))

        # ---- persistent pools ----
        const = ctx.enter_context(tc.tile_pool(name="const", bufs=1))
        vpool = ctx.enter_context(tc.tile_pool(name="vsb", bufs=1))
        opool = ctx.enter_context(tc.tile_pool(name="osb", bufs=1))
        kpool = ctx.enter_context(tc.tile_pool(name="ksb", bufs=3))
        qpool = ctx.enter_context(tc.tile_pool(name="qsb", bufs=3))
        ppool = ctx.enter_context(tc.tile_pool(name="pex", bufs=10))
        rpool = ctx.enter_context(tc.tile_pool(name="recip", bufs=2))
        bpool = ctx.enter_context(tc.tile_pool(name="bcast", bufs=2))
        drp = ctx.enter_context(tc.tile_pool(name="rscratch", bufs=4, space="DRAM"))
        outsb = ctx.enter_context(tc.tile_pool(name="outsb", bufs=3))
        xp = ctx.enter_context(tc.tile_pool(name="x", bufs=1))
        wqp = ctx.enter_context(tc.tile_pool(name="wq", bufs=1))
        wkp = ctx.enter_context(tc.tile_pool(name="wk", bufs=1))
        wvp = ctx.enter_context(tc.tile_pool(name="wv", bufs=1))
        wpp = ctx.enter_context(tc.tile_pool(name="wp", bufs=1))
        mm_ps = ctx.enter_context(tc.tile_pool(name="mm_ps", bufs=2, space="PSUM"))
        st_ps = ctx.enter_context(tc.tile_pool(name="st_ps", bufs=2, space="PSUM"))
        pv_ps = mm_ps

        # single mega-tiles: each input lands in ONE coalesced DMA descriptor
        # (a dma_start costs ~620ns of queue issue time; 61 descriptors would
        # serialize for ~38us on the sync engine)
        xt_all = xp.tile([128, 8, T], BF16, name="xt_all")
        wq_all = wqp.tile([128, 8, 512], BF16, name="wq_all")
        wk_all = wkp.tile([128, 8, 512], BF16, name="wk_all")
        wv_all = wvp.tile([128, 8, 512], BF16, name="wv_all")
        wp_all = wpp.tile([128, 4, D], BF16, name="wp_all")
        xt = [xt_all[:, d, :] for d in range(8)]
        wq = [wq_all[:, d, :] for d in range(8)]
        wk = [wk_all[:, d, :] for d in range(8)]
        wv = [wv_all[:, d, :] for d in range(8)]
        wp = [wp_all[:, e, :] for e in range(4)]

        # multiplicative post-exp causal staircase, replicated for both heads
        mask_sb = const.tile([128, 2, 128], BF16)
        nc.sync.dma_start(
            out=mask_sb[:, :, :], in_=maskd[:, :].rearrange("p (h n) -> p h n", h=2))
        ones_sb = const.tile([1, 64], BF16)
        nc.vector.memset(ones_sb[0:1, :], 1.0)
        ident_sb = const.tile([128, 128], BF16)
        nc.sync.dma_start(out=ident_sb[:, :], in_=identd[:, :])

        V_sb = [vpool.tile([128, 8, DH + 1], BF16, tag=f"v{m}", name=f"v{m}")
                for m in range(KT)]
        for m in range(KT):
            nc.vector.memset(V_sb[m][:, :, DH:DH + 1], 1.0)
        O_sb = [opool.tile([128, T], BF16, tag=f"o{p}", name=f"o{p}")
                for p in range(NPAIR)]

        # ---- DMA order: mask first (feeds the warmup burst), wv + first x
        # chunk unblock V(0..3), then wk/wq for K0/Q0, then the rest ----
        xTg = x_T.rearrange("(g p) t -> p g t", p=128)
        nc.sync.dma_start(out=wv_all[:, :, :],
                          in_=w_vT.rearrange("(g p) t -> p g t", p=128))
        nc.sync.dma_start(out=xt_all[:, :, 0:512], in_=xTg[:, :, 0:512])
        nc.sync.dma_start(out=wk_all[:, :, :],
                          in_=w_kT.rearrange("(g p) t -> p g t", p=128))
        nc.sync.dma_start(out=wq_all[:, :, :],
                          in_=w_qT.rearrange("(g p) t -> p g t", p=128))
        for cc in range(1, 4):
            nc.sync.dma_start(out=xt_all[:, :, 512 * cc:512 * cc + 512],
                              in_=xTg[:, :, 512 * cc:512 * cc + 512])
        nc.sync.dma_start(out=wp_all[:, :, :],
                          in_=w_pT.rearrange("(g p) t -> p g t", p=128))

        # ---- HAM warmup: ~3.5us of throwaway matmuls on the 64KB mask tile
        # (arrives within ~1us) so the PE clock gate is at 8/8 before the
        # first real projection burst ----
        wup = st_ps.tile([128, 2, 512], F32, tag="st", name="warm")
        for w in range(16 if os.environ.get('SKIP_WARM','') != '1' else 0):
            nc.tensor.matmul(
                wup[:, 0, 0:256],
                lhsT=ident_sb[:, :], rhs=mask_sb[:, :, :],
                start=True, stop=True,
            )

        # ---- V projection (x stationary), strided into V_sb; emitted in
        # head-halves so the pairs-2/3 half can run late as pair-1 filler ----
        def gen_v(m, h):
            ps = mm_ps.tile([128, 512], F32, tag="ps", name="ps")
            for d in range(8):
                nc.tensor.matmul(
                    ps[:, 0:256],
                    lhsT=xt[d][:, 128 * m:128 * m + 128],
                    rhs=wv[d][:, 256 * h:256 * h + 256],
                    start=(d == 0), stop=(d == 7),
                )
                if d < 7:
                    yield
            nc.scalar.copy(
                V_sb[m][:, 4 * h:4 * h + 4, 0:DH],
                ps[:, 0:256].rearrange("p (h e) -> p h e", h=4),
            )

        def gen_v_q(m, p):
            # single-pair V quarter (N=128): late filler for pairs 2/3
            ps = mm_ps.tile([128, 512], F32, tag="ps", name="ps")
            for d in range(8):
                nc.tensor.matmul(
                    ps[:, 0:128],
                    lhsT=xt[d][:, 128 * m:128 * m + 128],
                    rhs=wv[d][:, 128 * p:128 * p + 128],
                    start=(d == 0), stop=(d == 7),
                )
                if d < 7:
                    yield
            nc.scalar.copy(
                V_sb[m][:, 2 * p:2 * p + 2, 0:DH],
                ps[:, 0:128].rearrange("p (h e) -> p h e", h=2),
            )

        # ---- K/Q projection chunks: c in 0..3 -> K token-chunk c;
        # c in 4..7 -> Q token-chunk c-4 ----
        def gen_kq(p, K_t, Q_t, c):
            ps = mm_ps.tile([128, 512], F32, tag="ps", name="ps")
            ww, dst = (wk, K_t) if c < 4 else (wq, Q_t)
            n = c % 4
            for d in range(8):
                nc.tensor.matmul(
                    ps[:, :],
                    lhsT=ww[d][:, 128 * p:128 * p + 128],
                    rhs=xt[d][:, 512 * n:512 * n + 512],
                    start=(d == 0), stop=(d == 7),
                )
                if d < 7:
                    yield
            nc.vector.tensor_copy(dst[:, 512 * n:512 * n + 512], ps[:, :])

        # ---- output projection, split to feed the tail pairs' dry units:
        # pairs 0-2's three matmuls per (m, chunk) run as pair-2 filler into a
        # bf16 SBUF partial; pair 3 re-injects it (identity matmul) and adds
        # its own term. This keeps the PE dense through pairs 2-3, where HAM
        # otherwise oscillates to half clock. ----
        part_sb = [[opool.tile([128, 512], BF16, tag=f"pt{n}_{m}", name=f"pt{n}_{m}")
                    for m in range(8)] for n in range(4)]

        def gen_part_m(m, n):
            ps = mm_ps.tile([128, 512], F32, tag="ps", name="ps")
            for p in range(3):
                nc.tensor.matmul(
                    ps[:, :],
                    lhsT=wp[p][:, 128 * m:128 * m + 128],
                    rhs=O_sb[p][:, 512 * n:512 * n + 512],
                    start=(p == 0), stop=(p == 2),
                )
                if p < 2:
                    yield
            nc.vector.tensor_copy(part_sb[n][m][:, :], ps[:, :])

        def gen_proj_m(m, n):
            ps = mm_ps.tile([128, 512], F32, tag="ps", name="ps")
            nc.tensor.matmul(
                ps[:, :], lhsT=ident_sb[:, :], rhs=part_sb[n][m][:, :],
                start=True, stop=False)
            yield
            nc.tensor.matmul(
                ps[:, :],
                lhsT=wp[3][:, 128 * m:128 * m + 128],
                rhs=O_sb[3][:, 512 * n:512 * n + 512],
                start=False, stop=True,
            )
            ob = outsb.tile([128, 512], BF16)
            # DVE, not ACT: the ACT engine saturates on exps in pair 3
            nc.vector.tensor_copy(ob[:, :], ps[:, :])
            nc.sync.dma_start(
                out=outd[128 * m:128 * m + 128, 512 * n:512 * n + 512],
                in_=ob[:, :],
            )

        def run_gen(g):
            for _ in g:
                pass

        def qk(K_t, Q_t, h_off, ki, q0, qw, st_out):
            nc.tensor.matmul(
                st_out,
                lhsT=K_t[h_off:h_off + 64, 128 * ki:128 * ki + 128],
                rhs=Q_t[h_off:h_off + 64, q0:q0 + qw],
                start=True, stop=True,
            )

        # startup burst: V 0..3 (pairs 0/1 half) then pair-0 K0/Q0 so J=0
        # starts ASAP
        for m in range(4):
            run_gen(gen_v(m, 0))
        KQ = {0: (kpool.tile([128, T], BF16, tag="k", name="k0"),
                  qpool.tile([128, T], BF16, tag="q", name="q0"))}
        run_gen(gen_kq(0, KQ[0][0], KQ[0][1], 0))
        run_gen(gen_kq(0, KQ[0][0], KQ[0][1], 4))

        # filler schedule: deadline (p, J) = must be fully emitted before that
        # pair/chunk's attention begins
        filler = Filler()
        for JJ in range(1, 4):
            filler.add((0, JJ), gen_kq(0, KQ[0][0], KQ[0][1], JJ))
            filler.add((0, JJ), gen_kq(0, KQ[0][0], KQ[0][1], JJ + 4))
            for m in range(4 * JJ, 4 * JJ + 4):
                filler.add((0, JJ), gen_v(m, 0))
        for p in range(1, NPAIR):
            KQ[p] = (kpool.tile([128, T], BF16, tag="k", name=f"k{p}"),
                     qpool.tile([128, T], BF16, tag="q", name=f"q{p}"))
            for JJ in range(4):
                filler.add((p, JJ), gen_kq(p, KQ[p][0], KQ[p][1], JJ))
                filler.add((p, JJ), gen_kq(p, KQ[p][0], KQ[p][1], JJ + 4))
                if p == 2:
                    # pairs 2/3's V half: consumed first at (2, JJ), so these
                    # 128 matmuls feed pair 1's otherwise-dry units
                    for m in range(4 * JJ, 4 * JJ + 4):
                        filler.add((2, JJ), gen_v(m, 1))

        for p in range(NPAIR):
            K_t, Q_t = KQ[p]
            for J in range(NJ):
                filler.drain_until((p, J))
                q0 = 512 * J
                nbulk = 4 * J
                nki = nbulk + 4
                # NOTE: tag "pv" (not "ps") -- the pvs accumulators live for a
                # whole J chunk while filler ps tiles rotate through the pool;
                # sharing slots would dead-stall the PE FIFO on a WAR hazard
                pvs = {}
                for hi in (0, 1):
                    pvs[hi] = pv_ps.tile([65, 512], F32, tag="pv", bufs=2, name="pv")

                # software-pipelined inner loop: QK one unit ahead, 4 filler
                # matmuls between, then the previous unit's PV -- the PE never
                # waits on an exp. One 2-bank st tile per k-tile holds BOTH
                # heads; the two K=64 QK matmuls run in concurrent row groups
                # and a single wide exp covers both heads.
                def emit_qk(ki):
                    e = ki - nbulk
                    qc0 = 0 if e < 0 else 128 * e
                    nw = 512 - qc0
                    st = st_ps.tile([128, 2, 512], F32, tag="st", name="st")
                    for hi, h_off in ((0, 0), (1, 64)):
                        qk(K_t, Q_t, h_off, ki, q0 + qc0, nw, st[:, hi, 0:nw])
                    pb = ppool.tile([128, 2, 512], BF16, tag="pb", name="pb")
                    nc.scalar.activation(pb[:, :, 0:nw], st[:, :, 0:nw], EXP)
                    if e >= 0:
                        nc.vector.tensor_mul(
                            pb[:, :, 0:128], pb[:, :, 0:128], mask_sb[:, :, :])
                    return pb, qc0, nw

                def emit_pv(ki, pb, qc0, nw):
                    for hi in (0, 1):
                        nc.tensor.matmul(
                            pvs[hi][:, qc0:qc0 + nw],
                            lhsT=V_sb[ki][:, 2 * p + hi, :],
                            rhs=pb[:, hi, 0:nw],
                            start=(ki == 0), stop=(ki == nki - 1),
                        )

                horizon = (p + 1, 3)
                prev = emit_qk(0)
                for ki in range(1, nki):
                    cur = emit_qk(ki)
                    if ki == 3 and p == 2 and J > 0:
                        # pairs 0-2 partial of proj chunk J-1 (O_sb[2] chunk
                        # J-1 completes ~3us into J via the normalize chain)
                        for m in range(8):
                            filler.add((2, 9), gen_part_m(m, J - 1))
                    if ki == 3 and p == 3 and J == 0:
                        for m in range(8):
                            filler.add((3, 1), gen_part_m(m, 3))
                    if ki == 3 and p == 3 and J > 0:
                        # final proj chunk J-1: identity-inject the partial,
                        # add pair 3's term
                        for m in range(8):
                            filler.add((3, 9), gen_proj_m(m, J - 1))
                    # 8-matmul pulls every other unit: same absorption as 4
                    # per unit but half the PE weight-source switches (each
                    # switch costs ~130ns of LDWEIGHTS/dispatch)
                    if ki % 2 == 1:
                        filler.pull(8, horizon)
                    emit_pv(ki - 1, *prev)
                    prev = cur
                filler.pull(4, horizon)
                emit_pv(nki - 1, *prev)

                # normalize: copy each pv tile to SBUF in one DVE op, freeing
                # the PSUM bank immediately (holding it through the DMA
                # round-trips below would head-of-line stall the PE FIFO).
                # Sums live in row 64; spread them [128,4] via DRAM so
                # reciprocal runs wide, then broadcast back across partitions.
                # The two heads' chains are interleaved so their DMA hop
                # latencies overlap instead of queueing serially.
                rt = {}
                for hi in (0, 1):
                    # [96,...] so rows 64:96 are addressable for the tail's
                    # block-transpose (only 0:65 are written)
                    rt[hi] = rpool.tile([96, 512], F32, name="rt")
                    nc.vector.tensor_copy(rt[hi][0:65, :], pvs[hi][:, :])

                if p == 3 and J == NJ - 1:
                    # tail-only normalize: the DMA spread/broadcast chain has
                    # ~2.5us latency per hop which is fully exposed on the
                    # final chunk. Instead: DVE 32x32 block-transpose puts the
                    # denominator row across partitions (cols 32b hold
                    # denom[32b+i]), reciprocal runs wide, transpose back
                    # yields a linear recip row, and an idle-PE ones-matmul
                    # broadcasts it across 64 partitions.
                    for hi in (0, 1):
                        td = rpool.tile([32, 512], F32, name="td")
                        nc.vector.tensor_copy(td[0:1, :], pvs[hi][64:65, :])
                        tt = rpool.tile([32, 512], F32, name="tt")
                        nc.vector.transpose(tt[:, :], td[:, :])
                        # bf16 from here: a K=1 fp32 matmul streams ~4x slower
                        tt2 = rpool.tile([32, 512], BF16, name="tt2")
                        with nc.allow_low_precision(
                                reason="bf16 softmax denominators are well "
                                       "inside the 2e-2 error budget"):
                            nc.vector.reciprocal(
                                tt2[:, :].rearrange("p (b j) -> p b j", j=32)[:, :, 0],
                                tt[:, :].rearrange("p (b j) -> p b j", j=32)[:, :, 0])
                        tb = rpool.tile([32, 512], BF16, name="tb")
                        nc.vector.transpose(tb[:, :], tt2[:, :])
                        bc_ps = mm_ps.tile([64, 512], F32, tag="ps", name="bcps")
                        nc.tensor.matmul(
                            bc_ps[:, :], lhsT=ones_sb[0:1, :], rhs=tb[0:1, :],
                            start=True, stop=True)
                        nc.vector.tensor_mul(
                            O_sb[p][64 * hi:64 * hi + 64, q0:q0 + 512],
                            rt[hi][0:64, :], bc_ps[:, :])
                    continue

                for hi in (0, 1):
                    rd = drp.tile([512], F32, name="rd")
                    nc.gpsimd.dma_start(out=rd[:], in_=rt[hi][64:65, :])
                    rs = rpool.tile([128, 4], F32, name="rs")
                    nc.gpsimd.dma_start(
                        out=rs[:, :], in_=rd.rearrange("(p f) -> p f", p=128))
                    rs2 = rpool.tile([128, 4], F32, name="rs2")
                    nc.vector.reciprocal(rs2[:, :], rs[:, :])
                    rd2 = drp.tile([512], F32, name="rd2")
                    nc.gpsimd.dma_start(
                        out=rd2.rearrange("(p f) -> p f", p=128), in_=rs2[:, :])
                    bc = bpool.tile([64, 512], F32, name="bc")
                    nc.gpsimd.dma_start(
                        out=bc[:, :],
                        in_=bass.AP(tensor=rd2.tensor, offset=rd2.offset,
                                    ap=[[0, 64]] + list(rd2.ap)),
                    )
                    nc.vector.tensor_mul(
                        O_sb[p][64 * hi:64 * hi + 64, q0:q0 + 512],
                        rt[hi][0:64, :], bc[:, :],
                    )

        # ---- tail: keep the PE busy and the clock gate warm through the
        # final normalize chain (otherwise HAM re-throttles and the last proj
        # chunk runs at half clock), then the last token chunk of proj ----
        wdn = st_ps.tile([128, 2, 512], F32, tag="st", name="warmdn")
        for w in range(28 if os.environ.get('SKIP_WARM','') != '1' else 0):
            nc.tensor.matmul(
                wdn[:, 0, 0:256], lhsT=ident_sb[:, :], rhs=mask_sb[:, 0:2, :],
                start=True, stop=True)
        filler.drain_all()
        for m in range(8):
            run_gen(gen_proj_m(m, 3))

    nc.finalize()
    return nc


def _host_inputs(x, W_qkv, W_proj):
    bf = ml_dtypes.bfloat16
    # per head-half slices, [in, out] layouts
    wq = [np.ascontiguousarray((W_qkv[512 * hh:512 * hh + 512] / 8.0).T.astype(bf))
          for hh in range(2)]
    wk = [np.ascontiguousarray(W_qkv[D + 512 * hh:D + 512 * hh + 512].T.astype(bf))
          for hh in range(2)]
    wv = [np.ascontiguousarray(W_qkv[2 * D + 512 * hh:2 * D + 512 * hh + 512].T.astype(bf))
          for hh in range(2)]
    wp = [np.ascontiguousarray(W_proj[:, 512 * hh:512 * hh + 512].T.astype(bf))
          for hh in range(2)]

    kk, qq = np.meshgrid(np.arange(128), np.arange(128), indexing="ij")
    stair = np.tile((kk <= qq).astype(np.float32), (1, 2)).astype(bf)
    ident = np.eye(128, dtype=np.float32).astype(bf)

    xT = [np.ascontiguousarray(x[b].T.astype(bf)) for b in range(B)]
    in_maps = []
    for c in range(NCORES):
        b, hh = c // 2, c % 2
        in_maps.append({
            "x_T": xT[b],
            "w_qT": wq[hh], "w_kT": wk[hh], "w_vT": wv[hh], "w_pT": wp[hh],
            "mask": stair, "ident": ident,
        })
    return in_maps


def _run(inputs, trace=False, trace_cores=None):
    if "nc" not in _cached:
        _cached["nc"] = _build_program()
    nc = _cached["nc"]
    in_maps = _host_inputs(inputs["x"], inputs["W_qkv"], inputs["W_proj"])
    res = run_bass_kernel_spmd(
        nc, in_maps, core_ids=list(range(NCORES)),
        trace=trace, trace_cores=trace_cores,
    )
    out = np.zeros((B, T, D), np.float32)
    for b in range(B):
        # unshard the head-sharded tensor-parallel layout: the two cores of a
        # batch hold complementary partial output projections; sum them
        oT = (res.results[2 * b]["out_T"].astype(np.float32)
              + res.results[2 * b + 1]["out_T"].astype(np.float32))
        out[b] = oT.T
    return out, res


def kernel(**inputs) -> np.ndarray:
    out, _ = _run(inputs, trace=os.environ.get("KERNEL_TRACE", "") == "1")
    return out
